# revision 18
# baseline (speedup 1.0000x reference)
"""Trainium2 Bass kernel for nn_DecoderLayerWithMOE (attention + dense MoE + FFN layer).

Sharding: 8 cores, zero collectives. Core c owns (batch b = c//2, s-half = c%2)
-> 1024 tokens. Each core computes K/V over the full sequence of its batch
(each batch's K/V projection is computed by its 2 cores redundantly), then
attention / MoE / FFN fully token-parallel. Host does slicing, weight
transposes, and the final gather. Host orders each core's sequence so its own
tokens are the first T columns (attention is permutation-invariant over keys).

On-chip layout: activations are kept transposed (feature dim on SBUF
partitions, tokens on the free dim) so every projection is a weight-stationary
matmul with the activation as the moving operand. Scores are computed as
S^T[k, q]; V is augmented with a ones-column so the ctx matmul (M=65) also
produces the softmax denominators. Partition-dim reductions / broadcasts
(layernorm stats, softmax sums, gate) run as tiny ones-vector matmuls on the
PE. Matmuls run as float32r (full-rate fp32, ~1.5e-4 rel err).
"""

import os
from contextlib import ExitStack

import numpy as np

# Full problem dims
S, B, D, H, E = 2048, 4, 1024, 16, 8
HD = D // H
F = 4 * D
NCORES = 8
P = 128
EPS = 1e-5


class Cfg:
    def __init__(self, D, Skv, T, H, E, F):
        self.D, self.Skv, self.T, self.H, self.E, self.F = D, Skv, T, H, E, F
        self.DT = D // P          # feature tiles
        self.KT = Skv // P        # key-token tiles
        self.CH = min(512, T)     # token chunk (moving N)
        self.NCH = T // self.CH
        self.SKC = Skv // self.CH
        self.FT = F // P
        assert H * 64 == D and F % (4 * P) == 0


FULL_CFG = Cfg(D=D, Skv=S, T=S * B // NCORES, H=H, E=E, F=F)


def build_program(cfg):
    import concourse.bacc as bacc
    import concourse.tile as tile
    import concourse.mybir as mybir

    f32 = mybir.dt.float32
    f32r = mybir.dt.float32r
    Al = mybir.AluOpType
    Af = mybir.ActivationFunctionType

    DT, KT, CH, NCH, SKC, FT = cfg.DT, cfg.KT, cfg.CH, cfg.NCH, cfg.SKC, cfg.FT
    Dd, Skv, T, Hh, Ee, Ff = cfg.D, cfg.Skv, cfg.T, cfg.H, cfg.E, cfg.F
    FH = FT // 2  # f-tiles per FFN half

    nc = bacc.Bacc("TRN2", target_bir_lowering=False, debug=False,
                   num_devices=NCORES)

    def din(name, shape):
        return nc.dram_tensor(name, list(shape), f32, kind="ExternalInput")

    xt = din("xt", (Dd, Skv))
    wqT = din("wqT", (Dd, Dd)); wkT = din("wkT", (Dd, Dd)); wvT = din("wvT", (Dd, Dd))
    woT = din("woT", (Dd, Dd)); gwT = din("gwT", (Dd, Ee))
    ewT = din("ewT", (Ee, Dd, Dd))
    w1T = din("w1T", (Dd, Ff)); w2T = din("w2T", (Ff, Dd))
    bqp = din("bqp", (P, DT)); bkp = din("bkp", (P, DT))
    bvb_d = din("bvb", (P, Dd)); bop = din("bop", (P, DT))
    gb8_d = din("gb8", (Ee, 1)); ebp = din("ebp", (P, Ee * DT))
    b1p = din("b1p", (P, FT)); b2p = din("b2p", (P, DT))
    gba = [din(f"gba{i}", (2, Dd)) for i in range(3)]
    gpa = [din(f"gpa{i}", (P, DT)) for i in range(3)]
    out_d = nc.dram_tensor("out", [Dd, T], f32, kind="ExternalOutput")
    qdram = nc.dram_tensor("qdram", [Dd, T], f32r)
    kdram = nc.dram_tensor("kdram", [Dd, Skv], f32r)
    vdram = nc.dram_tensor("vdram", [Hh, Skv, 65], f32r)

    def r(ap):  # f32r view of a dram fp32 AP
        return ap.bitcast(f32r)

    def wload(pool, tag, bufs, name, dram_ap, ni, width):
        """Load ni stacked [P, width] i-tiles of a (ni*P, width) dram slice
        into one [P, ni*width] tile (one DMA)."""
        t = pool.tile([P, ni * width], f32r, tag=tag, bufs=bufs, name=name)
        nc.sync.dma_start(
            t[:].rearrange("p (i o) -> p i o", o=width),
            r(dram_ap).rearrange("(i p) o -> p i o", p=P))
        return t

    with ExitStack() as top:
        top.enter_context(nc.allow_low_precision(
            reason="float32r is bit-identical to fp32; PE rounds internally"))
        tc = top.enter_context(tile.TileContext(nc))
        pers = top.enter_context(tc.tile_pool(name="pers", bufs=1))
        pmm = top.enter_context(tc.tile_pool(name="pmm", bufs=1, space="PSUM"))
        pbc = top.enter_context(tc.tile_pool(name="pbc", bufs=1, space="PSUM"))
        psm = top.enter_context(tc.tile_pool(name="psm", bufs=1))

        # ---------- persistent small tensors ----------
        ones_m = pers.tile([P, max(P, CH)], f32, name="ones_m")
        nc.vector.memset(ones_m[:], 1.0)
        ones_col = pers.tile([P, 1], f32r, name="ones_col")
        nc.vector.tensor_copy(ones_col[:], ones_m[:, 0:1])
        ones_row = pers.tile([1, P], f32r, name="ones_row")
        nc.vector.tensor_copy(ones_row[:], ones_m[0:1, 0:P])
        bq_t = pers.tile([P, DT], f32, name="bq_t")
        nc.sync.dma_start(bq_t[:], bqp[:, :])
        bk_t = pers.tile([P, DT], f32, name="bk_t")
        nc.sync.dma_start(bk_t[:], bkp[:, :])
        bo_t = pers.tile([P, DT], f32, name="bo_t")
        nc.sync.dma_start(bo_t[:], bop[:, :])
        gb8_t = pers.tile([P, 1], f32, name="gb8_t")
        nc.sync.dma_start(gb8_t[0:Ee, :], gb8_d[:, :])
        eb_t = pers.tile([P, Ee * DT], f32, name="eb_t")
        nc.sync.dma_start(eb_t[:], ebp[:, :])
        b1_t = pers.tile([P, FT], f32, name="b1_t")
        nc.sync.dma_start(b1_t[:], b1p[:, :])
        b2_t = pers.tile([P, DT], f32, name="b2_t")
        nc.sync.dma_start(b2_t[:], b2p[:, :])
        gp_t = []
        for i in range(3):
            g2 = pers.tile([P, DT], f32, name=f"gp_t{i}")
            nc.sync.dma_start(g2[:], gpa[i][:, :])
            gp_t.append(g2)

        # ---------- generic layernorm over DT tiles of [P, T] ----------
        def layer_norm(src, dst, ln_idx, ptr):
            gbt = ptr.tile([2, Dd], f32r, tag="gb", bufs=1, name=f"gb_{ln_idx}")
            nc.sync.dma_start(gbt[:], r(gba[ln_idx][:, :]))
            gbx = gbt[:]
            gpx = gp_t[ln_idx]
            invD = 1.0 / Dd
            for c in range(NCH):
                cs = slice(c * CH, (c + 1) * CH)
                sum_ps = pmm.tile([P, CH], f32, tag="mm", bufs=2, name="ln_sum")
                sq_ps = pmm.tile([P, CH], f32, tag="mm2", bufs=1, name="ln_sq")
                for i in range(DT):
                    sq = ptr.tile([P, CH], f32r, tag="sq", bufs=3, name="ln_sqt")
                    nc.vector.tensor_tensor(
                        sq[:], src[i][:, cs].bitcast(f32), src[i][:, cs].bitcast(f32),
                        op=Al.mult)
                    nc.tensor.matmul(sum_ps[0:1, :], ones_col[:, :], src[i][:, cs],
                                     start=(i == 0), stop=(i == DT - 1))
                    nc.tensor.matmul(sq_ps[0:1, :], ones_col[:, :], sq[:],
                                     start=(i == 0), stop=(i == DT - 1))
                mu = psm.tile([1, CH], f32, tag="mu", name="ln_mu")
                var = psm.tile([1, CH], f32, tag="var", name="ln_var")
                tmp = psm.tile([1, CH], f32, tag="tmp", name="ln_tmp")
                nc.vector.tensor_scalar_mul(mu[:], sum_ps[0:1, :], invD)
                nc.vector.tensor_scalar_mul(var[:], sq_ps[0:1, :], invD)
                nc.vector.tensor_tensor(tmp[:], mu[:], mu[:], op=Al.mult)
                nc.vector.tensor_tensor(var[:], var[:], tmp[:], op=Al.subtract)
                nc.vector.tensor_scalar_add(var[:], var[:], EPS)
                nc.scalar.sqrt(var[:], var[:])
                srow = psm.tile([1, CH], f32r, tag="srow", name="ln_srow")
                nc.vector.reciprocal(srow[:], var[:])
                so = psm.tile([2, CH], f32r, tag="so", name="ln_so")
                nc.vector.tensor_copy(so[0:2, :], ones_m[0:2, 0:CH])  # row1 stays ones
                nc.vector.tensor_tensor(so[0:1, :], mu[:], srow[:].bitcast(f32),
                                        op=Al.mult)
                nc.vector.tensor_scalar_mul(so[0:1, :], so[0:1, :].bitcast(f32), -1.0)
                sb_ps = pbc.tile([P, CH], f32, tag="bc", bufs=2, name="ln_sb")
                nc.tensor.matmul(sb_ps[:, :], ones_row[:, :], srow[:, :],
                                 start=True, stop=True)
                for i in range(DT):
                    og_ps = pbc.tile([P, CH], f32, tag="bc2", bufs=1, name="ln_og")
                    nc.tensor.matmul(og_ps[:, :], gbx[:, i * P:(i + 1) * P], so[:, :],
                                     start=True, stop=True)
                    t1 = ptr.tile([P, CH], f32, tag="t1", bufs=3, name="ln_t1")
                    nc.vector.tensor_tensor(t1[:], src[i][:, cs].bitcast(f32),
                                            sb_ps[:, :], op=Al.mult)
                    nc.vector.scalar_tensor_tensor(
                        dst[i][:, cs], t1[:], gpx[:, i:i + 1], og_ps[:, :],
                        op0=Al.mult, op1=Al.add)

        # ================= Phase 1: QKV =================
        with tc.tile_pool(name="pxt", bufs=1) as pxt:
            bvb = pxt.tile([P, Dd], f32, name="bvb")
            nc.sync.dma_start(bvb[:], bvb_d[:, :])
            xt_t = []
            for i in range(DT):
                xx = pxt.tile([P, Skv], f32r, name=f"xt{i}")
                nc.sync.dma_start(xx[:], r(xt[i * P:(i + 1) * P, :]))
                xt_t.append(xx)

            with tc.tile_pool(name="pqk", bufs=1) as pqk:
                for j in range(DT):
                    wq = wload(pqk, "wq", 2, f"wq{j}", wqT[:, j * P:(j + 1) * P], DT, P)
                    wk = wload(pqk, "wk", 2, f"wk{j}", wkT[:, j * P:(j + 1) * P], DT, P)
                    for c in range(NCH):
                        ps = pmm.tile([P, CH], f32, tag="mm", bufs=2, name="q_ps")
                        for i in range(DT):
                            nc.tensor.matmul(ps[:, :], wq[:, i * P:(i + 1) * P],
                                             xt_t[i][:, c * CH:(c + 1) * CH],
                                             start=(i == 0), stop=(i == DT - 1))
                        qb = pqk.tile([P, CH], f32r, tag="qb", bufs=2, name="q_bounce")
                        nc.vector.tensor_scalar_add(qb[:], ps[:, :], bq_t[:, j:j + 1])
                        nc.sync.dma_start(
                            qdram[j * P:(j + 1) * P, c * CH:(c + 1) * CH], qb[:])
                    for c in range(SKC):
                        ps = pmm.tile([P, CH], f32, tag="mm", bufs=2, name="k_ps")
                        for i in range(DT):
                            nc.tensor.matmul(ps[:, :], wk[:, i * P:(i + 1) * P],
                                             xt_t[i][:, c * CH:(c + 1) * CH],
                                             start=(i == 0), stop=(i == DT - 1))
                        kb = pqk.tile([P, CH], f32r, tag="kb", bufs=2, name="k_bounce")
                        nc.vector.tensor_scalar_add(kb[:], ps[:, :], bk_t[:, j:j + 1])
                        nc.sync.dma_start(
                            kdram[j * P:(j + 1) * P, c * CH:(c + 1) * CH], kb[:])

            # V (activation-stationary, weight-moving) -> vdram augmented
            HPC = CH // 64  # heads per o-chunk
            with tc.tile_pool(name="pvv", bufs=1) as pvv:
                for oc in range(Dd // CH):
                    wv = wload(pvv, "wv", 2, f"wv{oc}", wvT[:, oc * CH:(oc + 1) * CH],
                               DT, CH)
                    for tt in range(KT):
                        ps = pmm.tile([P, CH], f32, tag="mm", bufs=2, name="v_ps")
                        for i in range(DT):
                            nc.tensor.matmul(ps[:, :], xt_t[i][:, tt * P:(tt + 1) * P],
                                             wv[:, i * CH:(i + 1) * CH],
                                             start=(i == 0), stop=(i == DT - 1))
                        vb = pvv.tile([P, HPC * 65], f32r, tag="vb", bufs=3, name="v_bounce")
                        vb3 = vb[:].rearrange("p (h c) -> p h c", c=65)
                        nc.vector.tensor_copy(
                            vb3[:, :, 64:65],
                            ones_m[:, 0:HPC].unsqueeze(2))
                        nc.vector.tensor_tensor(
                            vb3[:, :, 0:64],
                            ps[:, :].rearrange("p (h c) -> p h c", c=64),
                            bvb[:, oc * CH:(oc + 1) * CH].rearrange("p (h c) -> p h c", c=64),
                            op=Al.add)
                        h0 = oc * HPC
                        nc.sync.dma_start(
                            vdram[h0:h0 + HPC, tt * P:(tt + 1) * P, :].transpose([1, 0, 2]),
                            vb3[:, :, :])

        # ============ Phase 2: attention ============
        es_attn = ExitStack()
        pattn = es_attn.enter_context(
            tc.tile_pool(name="pattn", bufs=1, side="right"))
        if True:
            ctx_t = [pattn.tile([P, T], f32r, tag=f"ctx{j}", name=f"ctx{j}")
                     for j in range(DT)]
            for h in range(Hh):
                j, half = h // 2, (h % 2) * 64
                qh = pattn.tile([P, T], f32r, tag="qh", bufs=2, name="qh")
                nc.sync.dma_start(qh[half:half + 64, :],
                                  qdram[j * P + half:j * P + half + 64, :])
                kh = pattn.tile([P, Skv], f32r, tag="kh", bufs=2, name="kh")
                nc.sync.dma_start(kh[half:half + 64, :],
                                  kdram[j * P + half:j * P + half + 64, :])
                vh = pattn.tile([P, KT * 65], f32r, tag="vh", bufs=2, name="vh")
                vh3 = vh[:].rearrange("p (k c) -> p k c", c=65)
                nc.sync.dma_start(vh3, vdram[h].rearrange("(k p) c -> p k c", p=P))
                for c in range(NCH):
                    cps = pmm.tile([P, CH], f32, tag="ctxps", bufs=2, name="ctx_ps")
                    for kt in range(KT):
                        sps = pmm.tile([P, CH], f32, tag="mm", bufs=2, name="s_ps")
                        nc.tensor.matmul(
                            sps[:, :],
                            kh[half:half + 64, kt * P:(kt + 1) * P],
                            qh[half:half + 64, c * CH:(c + 1) * CH],
                            start=True, stop=True)
                        pt = pattn.tile([P, CH], f32r, tag="pt", bufs=4, name="p_t")
                        nc.scalar.activation(pt[:], sps[:, :], Af.Exp, scale=0.125)
                        nc.tensor.matmul(cps[0:65, :], vh3[:, kt, :], pt[:],
                                         start=(kt == 0), stop=(kt == KT - 1))
                    rec = psm.tile([1, CH], f32r, tag="rec", bufs=2, name="rec")
                    nc.vector.reciprocal(rec[:], cps[64:65, :])
                    bc = pbc.tile([P, CH], f32, tag="bc", bufs=2, name="att_bc")
                    nc.tensor.matmul(bc[0:64, :], ones_row[:, 0:64], rec[:, :],
                                     start=True, stop=True)
                    bcs = pattn.tile([P, CH], f32, tag="bcs", bufs=2, name="att_bcs")
                    nc.scalar.copy(bcs[0:64, :], bc[0:64, :])
                    if half == 0:
                        nc.vector.tensor_tensor(
                            ctx_t[j][0:64, c * CH:(c + 1) * CH],
                            cps[0:64, :], bcs[0:64, :], op=Al.mult)
                    else:
                        tmp = pattn.tile([P, CH], f32r, tag="ctmp", bufs=2, name="ctx_tmp")
                        nc.vector.tensor_tensor(
                            tmp[0:64, :], cps[0:64, :], bcs[0:64, :], op=Al.mult)
                        nc.sync.dma_start(
                            ctx_t[j][64:128, c * CH:(c + 1) * CH], tmp[0:64, :])

            # ---- out-proj (+ residual & b_out) ----
            es_res = ExitStack()
            pres = es_res.enter_context(tc.tile_pool(name="pres", bufs=1))
            res_t = [pres.tile([P, T], f32r, name=f"res{o}") for o in range(DT)]
            with tc.tile_pool(name="pxr", bufs=1) as pxr:
                xr_t = []
                for o in range(DT):
                    xr = pxr.tile([P, T], f32r, name=f"xr{o}")
                    nc.sync.dma_start(xr[:], r(xt[o * P:(o + 1) * P, 0:T]))
                    xr_t.append(xr)
                for o in range(DT):
                    wo = wload(pattn, "wo", 2, f"wo{o}", woT[:, o * P:(o + 1) * P], DT, P)
                    for c in range(NCH):
                        ps = pmm.tile([P, CH], f32, tag="mm", bufs=2, name="ao_ps")
                        for i in range(DT):
                            nc.tensor.matmul(ps[:, :], wo[:, i * P:(i + 1) * P],
                                             ctx_t[i][:, c * CH:(c + 1) * CH],
                                             start=(i == 0), stop=(i == DT - 1))
                        nc.vector.scalar_tensor_tensor(
                            res_t[o][:, c * CH:(c + 1) * CH], ps[:, :],
                            bo_t[:, o:o + 1],
                            xr_t[o][:, c * CH:(c + 1) * CH].bitcast(f32),
                            op0=Al.add, op1=Al.add)
        es_attn.close()  # ctx/qh/vh/pt freed

        # ============ Phase 3: LN1, gate, MoE, LN2 ============
        es_x1 = ExitStack()
        px1 = es_x1.enter_context(tc.tile_pool(name="px1", bufs=1, side="right"))
        x1_t = [px1.tile([P, T], f32r, name=f"x1_{i}") for i in range(DT)]
        layer_norm(res_t, x1_t, 0, px1)
        es_res.close()

        es_moe = ExitStack()
        pmoe = es_moe.enter_context(tc.tile_pool(name="pmoe", bufs=1))
        pgate = pmoe.tile([P, T], f32r, name="pgate")
        for c in range(NCH):
            gl_ps = pmm.tile([P, CH], f32, tag="mm", bufs=2, name="gl_ps")
            for i in range(DT):
                gw = pmoe.tile([P, Ee], f32r, tag="gw", bufs=3, name=f"gw{c}_{i}")
                nc.sync.dma_start(gw[:], r(gwT[i * P:(i + 1) * P, :]))
                nc.tensor.matmul(gl_ps[0:Ee, :], gw[:],
                                 x1_t[i][:, c * CH:(c + 1) * CH],
                                 start=(i == 0), stop=(i == DT - 1))
            eg = pmoe.tile([P, CH], f32r, tag="eg", bufs=1, name="eg")
            nc.scalar.activation(eg[0:Ee, :], gl_ps[0:Ee, :], Af.Exp,
                                 bias=gb8_t[0:Ee, :])
            gs_ps = pmm.tile([P, CH], f32, tag="mm2", bufs=1, name="gs_ps")
            nc.tensor.matmul(gs_ps[0:1, :], ones_col[0:Ee, :], eg[0:Ee, :],
                             start=True, stop=True)
            grec = psm.tile([1, CH], f32r, tag="rec", bufs=2, name="grec")
            nc.vector.reciprocal(grec[:], gs_ps[0:1, :])
            gb_ps = pbc.tile([P, CH], f32, tag="bc", bufs=2, name="gb_ps")
            nc.tensor.matmul(gb_ps[0:Ee, :], ones_row[:, 0:Ee], grec[:, :],
                             start=True, stop=True)
            nc.vector.tensor_tensor(
                pgate[0:Ee, c * CH:(c + 1) * CH],
                eg[0:Ee, :].bitcast(f32), gb_ps[0:Ee, :], op=Al.mult)

        acc_t = [pmoe.tile([P, T], f32r, tag=f"acc{o}", name=f"acc{o}")
                 for o in range(DT)]
        for e in range(Ee):
            ge_ps = []
            for c in range(NCH):
                grow = pmoe.tile([1, CH], f32r, tag="grow", bufs=2, name=f"grow{e}_{c}")
                nc.sync.dma_start(grow[:], pgate[e:e + 1, c * CH:(c + 1) * CH])
                g = pbc.tile([P, CH], f32, tag="bc", bufs=2, name=f"ge{e}_{c}")
                nc.tensor.matmul(g[:, :], ones_row[:, :], grow[:, :],
                                 start=True, stop=True)
                ge_ps.append(g)
            for o in range(DT):
                we = wload(pmoe, "we", 2, f"we{e}_{o}",
                           ewT[e, :, o * P:(o + 1) * P], DT, P)
                for c in range(NCH):
                    ps = pmm.tile([P, CH], f32, tag="mm", bufs=2, name="moe_ps")
                    for i in range(DT):
                        nc.tensor.matmul(ps[:, :], we[:, i * P:(i + 1) * P],
                                         x1_t[i][:, c * CH:(c + 1) * CH],
                                         start=(i == 0), stop=(i == DT - 1))
                    he = pmoe.tile([P, CH], f32, tag="he", bufs=2, name="he")
                    nc.scalar.activation(he[:], ps[:, :], Af.Relu,
                                         bias=eb_t[:, e * DT + o:e * DT + o + 1])
                    cs = slice(c * CH, (c + 1) * CH)
                    if e == 0:
                        nc.vector.tensor_tensor(
                            acc_t[o][:, cs], he[:], ge_ps[c][:, :], op=Al.mult)
                    else:
                        hg = pmoe.tile([P, CH], f32, tag="hg", bufs=2, name="hg")
                        nc.vector.tensor_tensor(hg[:], he[:], ge_ps[c][:, :],
                                                op=Al.mult)
                        nc.vector.tensor_tensor(
                            acc_t[o][:, cs], acc_t[o][:, cs].bitcast(f32),
                            hg[:], op=Al.add)
        # resid2 = x1 + moe
        for o in range(DT):
            nc.vector.tensor_tensor(acc_t[o][:], acc_t[o][:].bitcast(f32),
                                    x1_t[o][:].bitcast(f32), op=Al.add)
        es_x1.close()

        es_ff = ExitStack()
        pff = es_ff.enter_context(tc.tile_pool(name="pff", bufs=1, side="right"))
        x2_t = [pff.tile([P, T], f32r, tag=f"x2_{i}", name=f"x2_{i}")
                for i in range(DT)]
        layer_norm(acc_t, x2_t, 1, pff)
        es_moe.close()

        # ============ Phase 4: FFN + LN3 ============
        fp_t = [pff.tile([P, T], f32r, tag=f"fp{o}", name=f"fp{o}")
                for o in range(DT)]
        FQ = FT // 4  # f-tiles per FFN quarter
        for fh in range(4):
            h_t = [pff.tile([P, T], f32r, tag=f"h{i2}", name=f"h{fh}_{i2}")
                   for i2 in range(FQ)]
            for o32 in range(FQ):
                o = fh * FQ + o32
                w1 = wload(pff, "w1", 2, f"w1_{o}", w1T[:, o * P:(o + 1) * P], DT, P)
                for c in range(NCH):
                    ps = pmm.tile([P, CH], f32, tag="mm", bufs=2, name="ff1_ps")
                    for i in range(DT):
                        nc.tensor.matmul(ps[:, :], w1[:, i * P:(i + 1) * P],
                                         x2_t[i][:, c * CH:(c + 1) * CH],
                                         start=(i == 0), stop=(i == DT - 1))
                    nc.scalar.activation(h_t[o32][:, c * CH:(c + 1) * CH], ps[:, :],
                                         Af.Relu, bias=b1_t[:, o:o + 1])
            for o in range(DT):
                w2 = wload(pff, "w2", 2, f"w2_{fh}_{o}",
                           w2T[fh * FQ * P:(fh + 1) * FQ * P, o * P:(o + 1) * P],
                           FQ, P)
                for c in range(NCH):
                    ps = pmm.tile([P, CH], f32, tag="mm", bufs=2, name="ff2_ps")
                    for i2 in range(FQ):
                        nc.tensor.matmul(ps[:, :], w2[:, i2 * P:(i2 + 1) * P],
                                         h_t[i2][:, c * CH:(c + 1) * CH],
                                         start=(i2 == 0), stop=(i2 == FQ - 1))
                    cs = slice(c * CH, (c + 1) * CH)
                    if fh == 0:
                        nc.vector.tensor_copy(fp_t[o][:, cs], ps[:, :])
                    elif fh < 3:
                        nc.vector.tensor_tensor(fp_t[o][:, cs],
                                                fp_t[o][:, cs].bitcast(f32),
                                                ps[:, :], op=Al.add)
                    else:
                        nc.vector.scalar_tensor_tensor(
                            fp_t[o][:, cs], ps[:, :], b2_t[:, o:o + 1],
                            fp_t[o][:, cs].bitcast(f32), op0=Al.add, op1=Al.add)
            if fh == 0:
                # fold the residual (x2) into the partial sum
                for o in range(DT):
                    nc.vector.tensor_tensor(fp_t[o][:], fp_t[o][:].bitcast(f32),
                                            x2_t[o][:].bitcast(f32), op=Al.add)
        # LN3 writes into the (now dead) x2 tiles, then out
        layer_norm(fp_t, x2_t, 2, pff)
        for o in range(DT):
            nc.sync.dma_start(out_d[o * P:(o + 1) * P, :], x2_t[o][:].bitcast(f32))
        es_ff.close()

    nc.compile()
    return nc


# ====================== host side ======================

def _pack_col(v, nt):
    # (nt*128,) -> (128, nt) partition-major
    return np.ascontiguousarray(np.asarray(v, np.float32).reshape(nt, P).T)


def make_weight_maps(w_in, b_in, w_out, b_out, gate_w, gate_b, exp_w, exp_b,
                     ffn_w1, ffn_b1, ffn_w2, ffn_b2, g1, be1, g2, be2, g3, be3,
                     cfg):
    Dd, Ee, FT, DT_ = cfg.D, cfg.E, cfg.FT, cfg.DT
    f = np.float32
    ct = np.ascontiguousarray
    m = {
        "wqT": ct(np.asarray(w_in, f)[0:Dd].T),
        "wkT": ct(np.asarray(w_in, f)[Dd:2 * Dd].T),
        "wvT": ct(np.asarray(w_in, f)[2 * Dd:3 * Dd].T),
        "woT": ct(np.asarray(w_out, f).T),
        "gwT": ct(np.asarray(gate_w, f).T),
        "ewT": ct(np.asarray(exp_w, f).transpose(0, 2, 1)),
        "w1T": ct(np.asarray(ffn_w1, f).T),
        "w2T": ct(np.asarray(ffn_w2, f).T),
        "bqp": _pack_col(np.asarray(b_in, f)[0:Dd], DT_),
        "bkp": _pack_col(np.asarray(b_in, f)[Dd:2 * Dd], DT_),
        "bvb": ct(np.broadcast_to(np.asarray(b_in, f)[2 * Dd:3 * Dd], (P, Dd))),
        "bop": _pack_col(b_out, DT_),
        "gb8": np.asarray(gate_b, f).reshape(Ee, 1),
        "ebp": ct(np.asarray(exp_b, f).reshape(Ee * DT_, P).T),
        "b1p": _pack_col(ffn_b1, FT),
        "b2p": _pack_col(ffn_b2, DT_),
        "gba0": ct(np.stack([g1, be1]).astype(f)),
        "gpa0": _pack_col(g1, DT_),
        "gba1": ct(np.stack([g2, be2]).astype(f)),
        "gpa1": _pack_col(g2, DT_),
        "gba2": ct(np.stack([g3, be3]).astype(f)),
        "gpa2": _pack_col(g3, DT_),
    }
    return m


_NC_CACHE = {}


def kernel(x, w_in, b_in, w_out, b_out, gate_w, gate_b, exp_w, exp_b,
           ffn_w1, ffn_b1, ffn_w2, ffn_b2, g1, be1, g2, be2, g3, be3):
    from concourse.bass_utils import run_bass_kernel_spmd

    cfg = FULL_CFG
    x = np.asarray(x, np.float32)
    wm = make_weight_maps(w_in, b_in, w_out, b_out, gate_w, gate_b, exp_w,
                          exp_b, ffn_w1, ffn_b1, ffn_w2, ffn_b2,
                          g1, be1, g2, be2, g3, be3, cfg)
    Th = cfg.T  # tokens per core (one s-half of one batch)
    in_maps = []
    for c in range(NCORES):
        b, half = c // 2, c % 2
        xb = x[:, b, :]                      # (S, D)
        own = xb[half * Th:(half + 1) * Th]  # (T, D)
        other = xb[(1 - half) * Th:(2 - half) * Th]
        xt_c = np.ascontiguousarray(
            np.concatenate([own, other], axis=0).T)  # (D, Skv), own first
        in_maps.append({**wm, "xt": xt_c})

    if "nc" not in _NC_CACHE:
        _NC_CACHE["nc"] = build_program(cfg)
    nc = _NC_CACHE["nc"]

    trace = bool(int(os.environ.get("KERNEL_TRACE", "0")))
    res = run_bass_kernel_spmd(nc, in_maps, core_ids=list(range(NCORES)),
                               trace=trace)
    _NC_CACHE["last_results"] = res

    out = np.empty((S, B, D), np.float32)
    for c in range(NCORES):
        b, half = c // 2, c % 2
        out[half * Th:(half + 1) * Th, b, :] = res.results[c]["out"].T
    return out


# revision 22
# speedup vs baseline: 84.1092x; 84.1092x over previous
"""Trainium2 Bass kernel for nn_DecoderLayerWithMOE (attention + dense MoE + FFN layer).

Sharding: 8 cores, zero collectives. Core c owns (batch b = c//2, s-half = c%2)
-> 1024 tokens. Each core computes K/V over the full sequence of its batch
(each batch's K/V projection is computed by its 2 cores redundantly), then
attention / MoE / FFN fully token-parallel. Host does slicing, weight
transposes, and the final gather. Host orders each core's sequence so its own
tokens are the first T columns (attention is permutation-invariant over keys).

On-chip layout: activations are kept transposed (feature dim on SBUF
partitions, tokens on the free dim) so every projection is a weight-stationary
matmul with the activation as the moving operand. Scores are computed as
S^T[k, q]; V is augmented with a ones-column so the ctx matmul (M=65) also
produces the softmax denominators. Partition-dim reductions / broadcasts
(layernorm stats, softmax sums, gate) run as tiny ones-vector matmuls on the
PE. Matmuls run as float32r (full-rate fp32, ~1.5e-4 rel err).
"""

import os
from contextlib import ExitStack

import numpy as np

# Full problem dims
S, B, D, H, E = 2048, 4, 1024, 16, 8
HD = D // H
F = 4 * D
NCORES = 8
P = 128
EPS = 1e-5


class Cfg:
    def __init__(self, D, Skv, T, H, E, F):
        self.D, self.Skv, self.T, self.H, self.E, self.F = D, Skv, T, H, E, F
        self.DT = D // P          # feature tiles
        self.KT = Skv // P        # key-token tiles
        self.CH = min(512, T)     # token chunk (moving N)
        self.NCH = T // self.CH
        self.SKC = Skv // self.CH
        self.FT = F // P
        assert H * 64 == D and F % (4 * P) == 0


FULL_CFG = Cfg(D=D, Skv=S, T=S * B // NCORES, H=H, E=E, F=F)


def build_program(cfg):
    import concourse.bacc as bacc
    import concourse.tile as tile
    import concourse.mybir as mybir

    f32 = mybir.dt.float32
    f32r = mybir.dt.float32r
    Al = mybir.AluOpType
    Af = mybir.ActivationFunctionType

    DT, KT, CH, NCH, SKC, FT = cfg.DT, cfg.KT, cfg.CH, cfg.NCH, cfg.SKC, cfg.FT
    Dd, Skv, T, Hh, Ee, Ff = cfg.D, cfg.Skv, cfg.T, cfg.H, cfg.E, cfg.F
    FH = FT // 2  # f-tiles per FFN half

    nc = bacc.Bacc("TRN2", target_bir_lowering=False, debug=False,
                   num_devices=NCORES)

    def din(name, shape):
        return nc.dram_tensor(name, list(shape), f32, kind="ExternalInput")

    xt = din("xt", (Dd, Skv))
    wqT = din("wqT", (Dd, Dd)); wkT = din("wkT", (Dd, Dd)); wvT = din("wvT", (Dd, Dd))
    woT = din("woT", (Dd, Dd)); gwT = din("gwT", (Dd, Ee))
    ewT = din("ewT", (Ee, Dd, Dd))
    w1T = din("w1T", (Dd, Ff)); w2T = din("w2T", (Ff, Dd))
    bqp = din("bqp", (P, DT)); bkp = din("bkp", (P, DT))
    bvb_d = din("bvb", (P, Dd)); bop = din("bop", (P, DT))
    gb8_d = din("gb8", (Ee, 1)); ebp = din("ebp", (P, Ee * DT))
    b1p = din("b1p", (P, FT)); b2p = din("b2p", (P, DT))
    gba = [din(f"gba{i}", (2, Dd)) for i in range(3)]
    gpa = [din(f"gpa{i}", (P, DT)) for i in range(3)]
    out_d = nc.dram_tensor("out", [Dd, T], f32, kind="ExternalOutput")
    qdram = nc.dram_tensor("qdram", [Dd, T], f32r)
    kdram = nc.dram_tensor("kdram", [Dd, Skv], f32r)
    vdram = nc.dram_tensor("vdram", [Hh, Skv, 65], f32r)

    def r(ap):  # f32r view of a dram fp32 AP
        return ap.bitcast(f32r)

    def wload(pool, tag, bufs, name, dram_ap, ni, width):
        """Load ni stacked [P, width] i-tiles of a (ni*P, width) dram slice
        into one [P, ni*width] tile (one DMA)."""
        t = pool.tile([P, ni * width], f32r, tag=tag, bufs=bufs, name=name)
        nc.sync.dma_start(
            t[:].rearrange("p (i o) -> p i o", o=width),
            r(dram_ap).rearrange("(i p) o -> p i o", p=P))
        return t

    with ExitStack() as top:
        top.enter_context(nc.allow_low_precision(
            reason="float32r is bit-identical to fp32; PE rounds internally"))
        tc = top.enter_context(tile.TileContext(nc))
        pers = top.enter_context(tc.tile_pool(name="pers", bufs=1))
        pmm = top.enter_context(tc.tile_pool(name="pmm", bufs=1, space="PSUM"))
        pbc = top.enter_context(tc.tile_pool(name="pbc", bufs=1, space="PSUM"))
        psm = top.enter_context(tc.tile_pool(name="psm", bufs=1))

        # ---------- persistent small tensors ----------
        ones_m = pers.tile([P, max(P, CH)], f32, name="ones_m")
        nc.vector.memset(ones_m[:], 1.0)
        ones_col = pers.tile([P, 1], f32r, name="ones_col")
        nc.vector.tensor_copy(ones_col[:], ones_m[:, 0:1])
        ones_row = pers.tile([1, P], f32r, name="ones_row")
        nc.vector.tensor_copy(ones_row[:], ones_m[0:1, 0:P])
        bq_t = pers.tile([P, DT], f32, name="bq_t")
        nc.sync.dma_start(bq_t[:], bqp[:, :])
        bk_t = pers.tile([P, DT], f32, name="bk_t")
        nc.sync.dma_start(bk_t[:], bkp[:, :])
        bo_t = pers.tile([P, DT], f32, name="bo_t")
        nc.sync.dma_start(bo_t[:], bop[:, :])
        gb8_t = pers.tile([P, 1], f32, name="gb8_t")
        nc.sync.dma_start(gb8_t[0:Ee, :], gb8_d[:, :])
        eb_t = pers.tile([P, Ee * DT], f32, name="eb_t")
        nc.sync.dma_start(eb_t[:], ebp[:, :])
        b1_t = pers.tile([P, FT], f32, name="b1_t")
        nc.sync.dma_start(b1_t[:], b1p[:, :])
        b2_t = pers.tile([P, DT], f32, name="b2_t")
        nc.sync.dma_start(b2_t[:], b2p[:, :])
        gp_t = []
        for i in range(3):
            g2 = pers.tile([P, DT], f32, name=f"gp_t{i}")
            nc.sync.dma_start(g2[:], gpa[i][:, :])
            gp_t.append(g2)

        # ---------- generic layernorm over DT tiles of [P, T] ----------
        def layer_norm(src, dst, ln_idx, ptr):
            gbt = ptr.tile([2, Dd], f32r, tag="gb", bufs=1, name=f"gb_{ln_idx}")
            nc.sync.dma_start(gbt[:], r(gba[ln_idx][:, :]))
            gbx = gbt[:]
            gpx = gp_t[ln_idx]
            invD = 1.0 / Dd
            for c in range(NCH):
                cs = slice(c * CH, (c + 1) * CH)
                sum_ps = pmm.tile([P, CH], f32, tag="mm", bufs=2, name="ln_sum")
                sq_ps = pmm.tile([P, CH], f32, tag="mm2", bufs=1, name="ln_sq")
                for i in range(DT):
                    sq = ptr.tile([P, CH], f32r, tag="sq", bufs=3, name="ln_sqt")
                    nc.vector.tensor_tensor(
                        sq[:], src[i][:, cs].bitcast(f32), src[i][:, cs].bitcast(f32),
                        op=Al.mult)
                    nc.tensor.matmul(sum_ps[0:1, :], ones_col[:, :], src[i][:, cs],
                                     start=(i == 0), stop=(i == DT - 1))
                    nc.tensor.matmul(sq_ps[0:1, :], ones_col[:, :], sq[:],
                                     start=(i == 0), stop=(i == DT - 1))
                mu = psm.tile([1, CH], f32, tag="mu", name="ln_mu")
                var = psm.tile([1, CH], f32, tag="var", name="ln_var")
                tmp = psm.tile([1, CH], f32, tag="tmp", name="ln_tmp")
                nc.vector.tensor_scalar_mul(mu[:], sum_ps[0:1, :], invD)
                nc.vector.tensor_scalar_mul(var[:], sq_ps[0:1, :], invD)
                nc.vector.tensor_tensor(tmp[:], mu[:], mu[:], op=Al.mult)
                nc.vector.tensor_tensor(var[:], var[:], tmp[:], op=Al.subtract)
                nc.vector.tensor_scalar_add(var[:], var[:], EPS)
                nc.scalar.sqrt(var[:], var[:])
                srow = psm.tile([1, CH], f32r, tag="srow", name="ln_srow")
                nc.vector.reciprocal(srow[:], var[:])
                so = psm.tile([2, CH], f32r, tag="so", name="ln_so")
                nc.vector.tensor_copy(so[0:2, :], ones_m[0:2, 0:CH])  # row1 stays ones
                nc.vector.tensor_tensor(so[0:1, :], mu[:], srow[:].bitcast(f32),
                                        op=Al.mult)
                nc.vector.tensor_scalar_mul(so[0:1, :], so[0:1, :].bitcast(f32), -1.0)
                sb_ps = pbc.tile([P, CH], f32, tag="bc", bufs=2, name="ln_sb")
                nc.tensor.matmul(sb_ps[:, :], ones_row[:, :], srow[:, :],
                                 start=True, stop=True)
                for i in range(DT):
                    og_ps = pbc.tile([P, CH], f32, tag="bc2", bufs=1, name="ln_og")
                    nc.tensor.matmul(og_ps[:, :], gbx[:, i * P:(i + 1) * P], so[:, :],
                                     start=True, stop=True)
                    t1 = ptr.tile([P, CH], f32, tag="t1", bufs=3, name="ln_t1")
                    nc.vector.tensor_tensor(t1[:], src[i][:, cs].bitcast(f32),
                                            sb_ps[:, :], op=Al.mult)
                    nc.vector.scalar_tensor_tensor(
                        dst[i][:, cs], t1[:], gpx[:, i:i + 1], og_ps[:, :],
                        op0=Al.mult, op1=Al.add)

        # ================= Phase 1: QKV =================
        with tc.tile_pool(name="pxt", bufs=1) as pxt:
            bvb = pxt.tile([P, Dd], f32, name="bvb")
            nc.sync.dma_start(bvb[:], bvb_d[:, :])
            xt_t = []
            for i in range(DT):
                xx = pxt.tile([P, Skv], f32r, name=f"xt{i}")
                nc.sync.dma_start(xx[:], r(xt[i * P:(i + 1) * P, :]))
                xt_t.append(xx)

            with tc.tile_pool(name="pqk", bufs=1) as pqk:
                for j in range(DT):
                    wq = wload(pqk, "wq", 2, f"wq{j}", wqT[:, j * P:(j + 1) * P], DT, P)
                    wk = wload(pqk, "wk", 2, f"wk{j}", wkT[:, j * P:(j + 1) * P], DT, P)
                    for c in range(NCH):
                        ps = pmm.tile([P, CH], f32, tag="mm", bufs=2, name="q_ps")
                        for i in range(DT):
                            nc.tensor.matmul(ps[:, :], wq[:, i * P:(i + 1) * P],
                                             xt_t[i][:, c * CH:(c + 1) * CH],
                                             start=(i == 0), stop=(i == DT - 1))
                        qb = pqk.tile([P, CH], f32r, tag="qb", bufs=2, name="q_bounce")
                        nc.vector.tensor_scalar_add(qb[:], ps[:, :], bq_t[:, j:j + 1])
                        nc.sync.dma_start(
                            qdram[j * P:(j + 1) * P, c * CH:(c + 1) * CH], qb[:])
                    for c in range(SKC):
                        ps = pmm.tile([P, CH], f32, tag="mm", bufs=2, name="k_ps")
                        for i in range(DT):
                            nc.tensor.matmul(ps[:, :], wk[:, i * P:(i + 1) * P],
                                             xt_t[i][:, c * CH:(c + 1) * CH],
                                             start=(i == 0), stop=(i == DT - 1))
                        kb = pqk.tile([P, CH], f32r, tag="kb", bufs=2, name="k_bounce")
                        nc.vector.tensor_scalar_add(kb[:], ps[:, :], bk_t[:, j:j + 1])
                        nc.sync.dma_start(
                            kdram[j * P:(j + 1) * P, c * CH:(c + 1) * CH], kb[:])

            # V (activation-stationary, weight-moving) -> vdram augmented
            HPC = CH // 64  # heads per o-chunk
            with tc.tile_pool(name="pvv", bufs=1) as pvv:
                for oc in range(Dd // CH):
                    wv = wload(pvv, "wv", 2, f"wv{oc}", wvT[:, oc * CH:(oc + 1) * CH],
                               DT, CH)
                    for tt in range(KT):
                        ps = pmm.tile([P, CH], f32, tag="mm", bufs=2, name="v_ps")
                        for i in range(DT):
                            nc.tensor.matmul(ps[:, :], xt_t[i][:, tt * P:(tt + 1) * P],
                                             wv[:, i * CH:(i + 1) * CH],
                                             start=(i == 0), stop=(i == DT - 1))
                        vb = pvv.tile([P, HPC * 65], f32r, tag="vb", bufs=3, name="v_bounce")
                        vb3 = vb[:].rearrange("p (h c) -> p h c", c=65)
                        nc.vector.tensor_copy(
                            vb3[:, :, 64:65],
                            ones_m[:, 0:HPC].unsqueeze(2))
                        nc.vector.tensor_tensor(
                            vb3[:, :, 0:64],
                            ps[:, :].rearrange("p (h c) -> p h c", c=64),
                            bvb[:, oc * CH:(oc + 1) * CH].rearrange("p (h c) -> p h c", c=64),
                            op=Al.add)
                        h0 = oc * HPC
                        nc.sync.dma_start(
                            vdram[h0:h0 + HPC, tt * P:(tt + 1) * P, :].transpose([1, 0, 2]),
                            vb3[:, :, :])

        # ============ Phase 2: attention ============
        es_attn = ExitStack()
        pattn = es_attn.enter_context(
            tc.tile_pool(name="pattn", bufs=1, side="right"))
        if True:
            ctx_t = [pattn.tile([P, T], f32r, tag=f"ctx{j}", name=f"ctx{j}")
                     for j in range(DT)]
            for j in range(Hh // 2):
                qh = pattn.tile([P, T], f32r, tag="qh", bufs=2, name="qh")
                nc.sync.dma_start(qh[:], qdram[j * P:(j + 1) * P, :])
                kh = pattn.tile([P, Skv], f32r, tag="kh", bufs=2, name="kh")
                nc.sync.dma_start(kh[:], kdram[j * P:(j + 1) * P, :])
                vhs = []
                for half in (0, 1):
                    vh = pattn.tile([P, KT * 65], f32r, tag=f"vh{half}",
                                    bufs=2, name=f"vh{half}")
                    vh3 = vh[:].rearrange("p (k c) -> p k c", c=65)
                    nc.sync.dma_start(
                        vh3, vdram[2 * j + half].rearrange("(k p) c -> p k c", p=P))
                    vhs.append(vh3)
                for c in range(NCH):
                    cps = [pmm.tile([P, CH], f32, tag="ctxps", bufs=2,
                                    name=f"ctx_ps{half}") for half in (0, 1)]
                    for kt in range(KT):
                        pts = []
                        for half in (0, 1):
                            hb = half * 64
                            sps = pmm.tile([P, CH], f32, tag="mm", bufs=2, name="s_ps")
                            nc.tensor.matmul(
                                sps[:, :],
                                kh[hb:hb + 64, kt * P:(kt + 1) * P],
                                qh[hb:hb + 64, c * CH:(c + 1) * CH],
                                start=True, stop=True)
                            pt = pattn.tile([P, CH], f32r, tag="pt", bufs=4, name="p_t")
                            nc.scalar.activation(pt[:], sps[:, :], Af.Exp, scale=0.125)
                            pts.append(pt)
                        for half in (0, 1):
                            nc.tensor.matmul(cps[half][0:65, :], vhs[half][:, kt, :],
                                             pts[half][:],
                                             start=(kt == 0), stop=(kt == KT - 1))
                    for half in (0, 1):
                        rec = psm.tile([1, CH], f32r, tag="rec", bufs=2, name="rec")
                        nc.vector.reciprocal(rec[:], cps[half][64:65, :])
                        bc = pbc.tile([P, CH], f32, tag="bc", bufs=2, name="att_bc")
                        nc.tensor.matmul(bc[0:64, :], ones_row[:, 0:64], rec[:, :],
                                         start=True, stop=True)
                        bcs = pattn.tile([P, CH], f32, tag="bcs", bufs=2, name="att_bcs")
                        nc.scalar.copy(bcs[0:64, :], bc[0:64, :])
                        if half == 0:
                            nc.vector.tensor_tensor(
                                ctx_t[j][0:64, c * CH:(c + 1) * CH],
                                cps[half][0:64, :], bcs[0:64, :], op=Al.mult)
                        else:
                            tmp = pattn.tile([P, CH], f32r, tag="ctmp", bufs=2,
                                             name="ctx_tmp")
                            nc.vector.tensor_tensor(
                                tmp[0:64, :], cps[half][0:64, :], bcs[0:64, :],
                                op=Al.mult)
                            nc.sync.dma_start(
                                ctx_t[j][64:128, c * CH:(c + 1) * CH], tmp[0:64, :])

            # ---- out-proj (+ residual & b_out) ----
            es_res = ExitStack()
            pres = es_res.enter_context(tc.tile_pool(name="pres", bufs=1))
            res_t = [pres.tile([P, T], f32r, name=f"res{o}") for o in range(DT)]
            with tc.tile_pool(name="pxr", bufs=1) as pxr:
                xr_t = []
                for o in range(DT):
                    xr = pxr.tile([P, T], f32r, name=f"xr{o}")
                    nc.sync.dma_start(xr[:], r(xt[o * P:(o + 1) * P, 0:T]))
                    xr_t.append(xr)
                for o in range(DT):
                    wo = wload(pattn, "wo", 2, f"wo{o}", woT[:, o * P:(o + 1) * P], DT, P)
                    for c in range(NCH):
                        ps = pmm.tile([P, CH], f32, tag="mm", bufs=2, name="ao_ps")
                        for i in range(DT):
                            nc.tensor.matmul(ps[:, :], wo[:, i * P:(i + 1) * P],
                                             ctx_t[i][:, c * CH:(c + 1) * CH],
                                             start=(i == 0), stop=(i == DT - 1))
                        nc.vector.scalar_tensor_tensor(
                            res_t[o][:, c * CH:(c + 1) * CH], ps[:, :],
                            bo_t[:, o:o + 1],
                            xr_t[o][:, c * CH:(c + 1) * CH].bitcast(f32),
                            op0=Al.add, op1=Al.add)
        es_attn.close()  # ctx/qh/vh/pt freed

        # ============ Phase 3: LN1, gate, MoE, LN2 ============
        es_x1 = ExitStack()
        px1 = es_x1.enter_context(tc.tile_pool(name="px1", bufs=1, side="right"))
        x1_t = [px1.tile([P, T], f32r, name=f"x1_{i}") for i in range(DT)]
        layer_norm(res_t, x1_t, 0, px1)
        es_res.close()

        es_moe = ExitStack()
        pmoe = es_moe.enter_context(tc.tile_pool(name="pmoe", bufs=1))
        pgate = pmoe.tile([P, T], f32r, name="pgate")
        for c in range(NCH):
            gl_ps = pmm.tile([P, CH], f32, tag="mm", bufs=2, name="gl_ps")
            for i in range(DT):
                gw = pmoe.tile([P, Ee], f32r, tag="gw", bufs=3, name=f"gw{c}_{i}")
                nc.sync.dma_start(gw[:], r(gwT[i * P:(i + 1) * P, :]))
                nc.tensor.matmul(gl_ps[0:Ee, :], gw[:],
                                 x1_t[i][:, c * CH:(c + 1) * CH],
                                 start=(i == 0), stop=(i == DT - 1))
            eg = pmoe.tile([P, CH], f32r, tag="eg", bufs=1, name="eg")
            nc.scalar.activation(eg[0:Ee, :], gl_ps[0:Ee, :], Af.Exp,
                                 bias=gb8_t[0:Ee, :])
            gs_ps = pmm.tile([P, CH], f32, tag="mm2", bufs=1, name="gs_ps")
            nc.tensor.matmul(gs_ps[0:1, :], ones_col[0:Ee, :], eg[0:Ee, :],
                             start=True, stop=True)
            grec = psm.tile([1, CH], f32r, tag="rec", bufs=2, name="grec")
            nc.vector.reciprocal(grec[:], gs_ps[0:1, :])
            gb_ps = pbc.tile([P, CH], f32, tag="bc", bufs=2, name="gb_ps")
            nc.tensor.matmul(gb_ps[0:Ee, :], ones_row[:, 0:Ee], grec[:, :],
                             start=True, stop=True)
            nc.vector.tensor_tensor(
                pgate[0:Ee, c * CH:(c + 1) * CH],
                eg[0:Ee, :].bitcast(f32), gb_ps[0:Ee, :], op=Al.mult)

        acc_t = [pmoe.tile([P, T], f32r, tag=f"acc{o}", name=f"acc{o}")
                 for o in range(DT)]
        for e in range(Ee):
            ge_ps = []
            for c in range(NCH):
                grow = pmoe.tile([1, CH], f32r, tag="grow", bufs=2, name=f"grow{e}_{c}")
                nc.sync.dma_start(grow[:], pgate[e:e + 1, c * CH:(c + 1) * CH])
                g = pbc.tile([P, CH], f32, tag="bc", bufs=2, name=f"ge{e}_{c}")
                nc.tensor.matmul(g[:, :], ones_row[:, :], grow[:, :],
                                 start=True, stop=True)
                ge_ps.append(g)
            for o in range(DT):
                we = wload(pmoe, "we", 2, f"we{e}_{o}",
                           ewT[e, :, o * P:(o + 1) * P], DT, P)
                for c in range(NCH):
                    ps = pmm.tile([P, CH], f32, tag="mm", bufs=2, name="moe_ps")
                    for i in range(DT):
                        nc.tensor.matmul(ps[:, :], we[:, i * P:(i + 1) * P],
                                         x1_t[i][:, c * CH:(c + 1) * CH],
                                         start=(i == 0), stop=(i == DT - 1))
                    he = pmoe.tile([P, CH], f32, tag="he", bufs=2, name="he")
                    nc.scalar.activation(he[:], ps[:, :], Af.Relu,
                                         bias=eb_t[:, e * DT + o:e * DT + o + 1])
                    cs = slice(c * CH, (c + 1) * CH)
                    if e == 0:
                        nc.vector.tensor_tensor(
                            acc_t[o][:, cs], he[:], ge_ps[c][:, :], op=Al.mult)
                    else:
                        hg = pmoe.tile([P, CH], f32, tag="hg", bufs=2, name="hg")
                        nc.vector.tensor_tensor(hg[:], he[:], ge_ps[c][:, :],
                                                op=Al.mult)
                        nc.vector.tensor_tensor(
                            acc_t[o][:, cs], acc_t[o][:, cs].bitcast(f32),
                            hg[:], op=Al.add)
        # resid2 = x1 + moe
        for o in range(DT):
            nc.vector.tensor_tensor(acc_t[o][:], acc_t[o][:].bitcast(f32),
                                    x1_t[o][:].bitcast(f32), op=Al.add)
        es_x1.close()

        es_ff = ExitStack()
        pff = es_ff.enter_context(tc.tile_pool(name="pff", bufs=1, side="right"))
        x2_t = [pff.tile([P, T], f32r, tag=f"x2_{i}", name=f"x2_{i}")
                for i in range(DT)]
        layer_norm(acc_t, x2_t, 1, pff)
        es_moe.close()

        # ============ Phase 4: FFN + LN3 ============
        fp_t = [pff.tile([P, T], f32r, tag=f"fp{o}", name=f"fp{o}")
                for o in range(DT)]
        FQ = FT // 4  # f-tiles per FFN quarter
        for fh in range(4):
            h_t = [pff.tile([P, T], f32r, tag=f"h{i2}", name=f"h{fh}_{i2}")
                   for i2 in range(FQ)]
            for o32 in range(FQ):
                o = fh * FQ + o32
                w1 = wload(pff, "w1", 2, f"w1_{o}", w1T[:, o * P:(o + 1) * P], DT, P)
                for c in range(NCH):
                    ps = pmm.tile([P, CH], f32, tag="mm", bufs=2, name="ff1_ps")
                    for i in range(DT):
                        nc.tensor.matmul(ps[:, :], w1[:, i * P:(i + 1) * P],
                                         x2_t[i][:, c * CH:(c + 1) * CH],
                                         start=(i == 0), stop=(i == DT - 1))
                    nc.scalar.activation(h_t[o32][:, c * CH:(c + 1) * CH], ps[:, :],
                                         Af.Relu, bias=b1_t[:, o:o + 1])
            for o in range(DT):
                w2 = wload(pff, "w2", 2, f"w2_{fh}_{o}",
                           w2T[fh * FQ * P:(fh + 1) * FQ * P, o * P:(o + 1) * P],
                           FQ, P)
                for c in range(NCH):
                    ps = pmm.tile([P, CH], f32, tag="mm", bufs=2, name="ff2_ps")
                    for i2 in range(FQ):
                        nc.tensor.matmul(ps[:, :], w2[:, i2 * P:(i2 + 1) * P],
                                         h_t[i2][:, c * CH:(c + 1) * CH],
                                         start=(i2 == 0), stop=(i2 == FQ - 1))
                    cs = slice(c * CH, (c + 1) * CH)
                    if fh == 0:
                        nc.vector.tensor_copy(fp_t[o][:, cs], ps[:, :])
                    elif fh < 3:
                        nc.vector.tensor_tensor(fp_t[o][:, cs],
                                                fp_t[o][:, cs].bitcast(f32),
                                                ps[:, :], op=Al.add)
                    else:
                        nc.vector.scalar_tensor_tensor(
                            fp_t[o][:, cs], ps[:, :], b2_t[:, o:o + 1],
                            fp_t[o][:, cs].bitcast(f32), op0=Al.add, op1=Al.add)
            if fh == 0:
                # fold the residual (x2) into the partial sum
                for o in range(DT):
                    nc.vector.tensor_tensor(fp_t[o][:], fp_t[o][:].bitcast(f32),
                                            x2_t[o][:].bitcast(f32), op=Al.add)
        # LN3 writes into the (now dead) x2 tiles, then out
        layer_norm(fp_t, x2_t, 2, pff)
        for o in range(DT):
            nc.sync.dma_start(out_d[o * P:(o + 1) * P, :], x2_t[o][:].bitcast(f32))
        es_ff.close()

    nc.compile()
    return nc


# ====================== host side ======================

def _pack_col(v, nt):
    # (nt*128,) -> (128, nt) partition-major
    return np.ascontiguousarray(np.asarray(v, np.float32).reshape(nt, P).T)


def make_weight_maps(w_in, b_in, w_out, b_out, gate_w, gate_b, exp_w, exp_b,
                     ffn_w1, ffn_b1, ffn_w2, ffn_b2, g1, be1, g2, be2, g3, be3,
                     cfg):
    Dd, Ee, FT, DT_ = cfg.D, cfg.E, cfg.FT, cfg.DT
    f = np.float32
    ct = np.ascontiguousarray
    m = {
        "wqT": ct(np.asarray(w_in, f)[0:Dd].T),
        "wkT": ct(np.asarray(w_in, f)[Dd:2 * Dd].T),
        "wvT": ct(np.asarray(w_in, f)[2 * Dd:3 * Dd].T),
        "woT": ct(np.asarray(w_out, f).T),
        "gwT": ct(np.asarray(gate_w, f).T),
        "ewT": ct(np.asarray(exp_w, f).transpose(0, 2, 1)),
        "w1T": ct(np.asarray(ffn_w1, f).T),
        "w2T": ct(np.asarray(ffn_w2, f).T),
        "bqp": _pack_col(np.asarray(b_in, f)[0:Dd], DT_),
        "bkp": _pack_col(np.asarray(b_in, f)[Dd:2 * Dd], DT_),
        "bvb": ct(np.broadcast_to(np.asarray(b_in, f)[2 * Dd:3 * Dd], (P, Dd))),
        "bop": _pack_col(b_out, DT_),
        "gb8": np.asarray(gate_b, f).reshape(Ee, 1),
        "ebp": ct(np.asarray(exp_b, f).reshape(Ee * DT_, P).T),
        "b1p": _pack_col(ffn_b1, FT),
        "b2p": _pack_col(ffn_b2, DT_),
        "gba0": ct(np.stack([g1, be1]).astype(f)),
        "gpa0": _pack_col(g1, DT_),
        "gba1": ct(np.stack([g2, be2]).astype(f)),
        "gpa1": _pack_col(g2, DT_),
        "gba2": ct(np.stack([g3, be3]).astype(f)),
        "gpa2": _pack_col(g3, DT_),
    }
    return m


_NC_CACHE = {}


def kernel(x, w_in, b_in, w_out, b_out, gate_w, gate_b, exp_w, exp_b,
           ffn_w1, ffn_b1, ffn_w2, ffn_b2, g1, be1, g2, be2, g3, be3):
    from concourse.bass_utils import run_bass_kernel_spmd

    cfg = FULL_CFG
    x = np.asarray(x, np.float32)
    wm = make_weight_maps(w_in, b_in, w_out, b_out, gate_w, gate_b, exp_w,
                          exp_b, ffn_w1, ffn_b1, ffn_w2, ffn_b2,
                          g1, be1, g2, be2, g3, be3, cfg)
    Th = cfg.T  # tokens per core (one s-half of one batch)
    in_maps = []
    for c in range(NCORES):
        b, half = c // 2, c % 2
        xb = x[:, b, :]                      # (S, D)
        own = xb[half * Th:(half + 1) * Th]  # (T, D)
        other = xb[(1 - half) * Th:(2 - half) * Th]
        xt_c = np.ascontiguousarray(
            np.concatenate([own, other], axis=0).T)  # (D, Skv), own first
        in_maps.append({**wm, "xt": xt_c})

    if "nc" not in _NC_CACHE:
        _NC_CACHE["nc"] = build_program(cfg)
    nc = _NC_CACHE["nc"]

    trace = bool(int(os.environ.get("KERNEL_TRACE", "0")))
    last_exc = None
    for attempt in range(3):
        try:
            res = run_bass_kernel_spmd(nc, in_maps, core_ids=list(range(NCORES)),
                                       trace=trace)
            break
        except Exception as e:  # transient axon/NRT hiccups — retry
            last_exc = e
            if attempt == 2:
                raise
    _NC_CACHE["last_results"] = res

    out = np.empty((S, B, D), np.float32)
    for c in range(NCORES):
        b, half = c // 2, c % 2
        out[half * Th:(half + 1) * Th, b, :] = res.results[c]["out"].T
    return out


# revision 24
# speedup vs baseline: 84.5426x; 1.0052x over previous
"""Trainium2 Bass kernel for nn_DecoderLayerWithMOE (attention + dense MoE + FFN layer).

Sharding: 8 cores, zero collectives. Core c owns (batch b = c//2, s-half = c%2)
-> 1024 tokens. Each core computes K/V over the full sequence of its batch
(each batch's K/V projection is computed by its 2 cores redundantly), then
attention / MoE / FFN fully token-parallel. Host does slicing, weight
transposes, and the final gather. Host orders each core's sequence so its own
tokens are the first T columns (attention is permutation-invariant over keys).

On-chip layout: activations are kept transposed (feature dim on SBUF
partitions, tokens on the free dim) so every projection is a weight-stationary
matmul with the activation as the moving operand. Scores are computed as
S^T[k, q]; V is augmented with a ones-column so the ctx matmul (M=65) also
produces the softmax denominators. Partition-dim reductions / broadcasts
(layernorm stats, softmax sums, gate) run as tiny ones-vector matmuls on the
PE. Matmuls run as float32r (full-rate fp32, ~1.5e-4 rel err).
"""

import os
from contextlib import ExitStack

import numpy as np

# Full problem dims
S, B, D, H, E = 2048, 4, 1024, 16, 8
HD = D // H
F = 4 * D
NCORES = 8
P = 128
EPS = 1e-5


class Cfg:
    def __init__(self, D, Skv, T, H, E, F):
        self.D, self.Skv, self.T, self.H, self.E, self.F = D, Skv, T, H, E, F
        self.DT = D // P          # feature tiles
        self.KT = Skv // P        # key-token tiles
        self.CH = min(512, T)     # token chunk (moving N)
        self.NCH = T // self.CH
        self.SKC = Skv // self.CH
        self.FT = F // P
        assert H * 64 == D and F % (4 * P) == 0


FULL_CFG = Cfg(D=D, Skv=S, T=S * B // NCORES, H=H, E=E, F=F)


def build_program(cfg):
    import concourse.bacc as bacc
    import concourse.tile as tile
    import concourse.mybir as mybir

    f32 = mybir.dt.float32
    f32r = mybir.dt.float32r
    Al = mybir.AluOpType
    Af = mybir.ActivationFunctionType

    DT, KT, CH, NCH, SKC, FT = cfg.DT, cfg.KT, cfg.CH, cfg.NCH, cfg.SKC, cfg.FT
    Dd, Skv, T, Hh, Ee, Ff = cfg.D, cfg.Skv, cfg.T, cfg.H, cfg.E, cfg.F
    FH = FT // 2  # f-tiles per FFN half

    nc = bacc.Bacc("TRN2", target_bir_lowering=False, debug=False,
                   num_devices=NCORES)

    def din(name, shape):
        return nc.dram_tensor(name, list(shape), f32, kind="ExternalInput")

    xt = din("xt", (Dd, Skv))
    wqT = din("wqT", (Dd, Dd)); wkT = din("wkT", (Dd, Dd)); wvT = din("wvT", (Dd, Dd))
    woT = din("woT", (Dd, Dd)); gwT = din("gwT", (Dd, Ee))
    ewT = din("ewT", (Ee, Dd, Dd))
    w1T = din("w1T", (Dd, Ff)); w2T = din("w2T", (Ff, Dd))
    bqp = din("bqp", (P, DT)); bkp = din("bkp", (P, DT))
    bvb_d = din("bvb", (P, Dd)); bop = din("bop", (P, DT))
    gb8_d = din("gb8", (Ee, 1)); ebp = din("ebp", (P, Ee * DT))
    b1p = din("b1p", (P, FT)); b2p = din("b2p", (P, DT))
    gba = [din(f"gba{i}", (2, Dd)) for i in range(3)]
    gpa = [din(f"gpa{i}", (P, DT)) for i in range(3)]
    out_d = nc.dram_tensor("out", [Dd, T], f32, kind="ExternalOutput")
    qdram = nc.dram_tensor("qdram", [Dd, T], f32r)
    kdram = nc.dram_tensor("kdram", [Dd, Skv], f32r)
    vdram = nc.dram_tensor("vdram", [Hh, Skv, 65], f32r)

    def r(ap):  # f32r view of a dram fp32 AP
        return ap.bitcast(f32r)

    def wload(pool, tag, bufs, name, dram_ap, ni, width):
        """Load ni stacked [P, width] i-tiles of a (ni*P, width) dram slice
        into one [P, ni*width] tile (one DMA)."""
        t = pool.tile([P, ni * width], f32r, tag=tag, bufs=bufs, name=name)
        nc.sync.dma_start(
            t[:].rearrange("p (i o) -> p i o", o=width),
            r(dram_ap).rearrange("(i p) o -> p i o", p=P))
        return t

    with ExitStack() as top:
        top.enter_context(nc.allow_low_precision(
            reason="float32r is bit-identical to fp32; PE rounds internally"))
        tc = top.enter_context(tile.TileContext(nc))
        pers = top.enter_context(tc.tile_pool(name="pers", bufs=1))
        pmm = top.enter_context(tc.tile_pool(name="pmm", bufs=1, space="PSUM"))
        pbc = top.enter_context(tc.tile_pool(name="pbc", bufs=1, space="PSUM"))
        psm = top.enter_context(tc.tile_pool(name="psm", bufs=1))

        # ---------- persistent small tensors ----------
        ones_m = pers.tile([P, max(P, CH)], f32, name="ones_m")
        nc.vector.memset(ones_m[:], 1.0)
        ones_col = pers.tile([P, 1], f32r, name="ones_col")
        nc.vector.tensor_copy(ones_col[:], ones_m[:, 0:1])
        ones_row = pers.tile([1, P], f32r, name="ones_row")
        nc.vector.tensor_copy(ones_row[:], ones_m[0:1, 0:P])
        bq_t = pers.tile([P, DT], f32, name="bq_t")
        nc.sync.dma_start(bq_t[:], bqp[:, :])
        bk_t = pers.tile([P, DT], f32, name="bk_t")
        nc.sync.dma_start(bk_t[:], bkp[:, :])
        bo_t = pers.tile([P, DT], f32, name="bo_t")
        nc.sync.dma_start(bo_t[:], bop[:, :])
        gb8_t = pers.tile([P, 1], f32, name="gb8_t")
        nc.sync.dma_start(gb8_t[0:Ee, :], gb8_d[:, :])
        eb_t = pers.tile([P, Ee * DT], f32, name="eb_t")
        nc.sync.dma_start(eb_t[:], ebp[:, :])
        b1_t = pers.tile([P, FT], f32, name="b1_t")
        nc.sync.dma_start(b1_t[:], b1p[:, :])
        b2_t = pers.tile([P, DT], f32, name="b2_t")
        nc.sync.dma_start(b2_t[:], b2p[:, :])
        gp_t = []
        for i in range(3):
            g2 = pers.tile([P, DT], f32, name=f"gp_t{i}")
            nc.sync.dma_start(g2[:], gpa[i][:, :])
            gp_t.append(g2)

        # ---------- generic layernorm over DT tiles of [P, T] ----------
        def layer_norm(src, dst, ln_idx, ptr):
            gbt = ptr.tile([2, Dd], f32r, tag="gb", bufs=1, name=f"gb_{ln_idx}")
            nc.sync.dma_start(gbt[:], r(gba[ln_idx][:, :]))
            gbx = gbt[:]
            gpx = gp_t[ln_idx]
            invD = 1.0 / Dd
            for c in range(NCH):
                cs = slice(c * CH, (c + 1) * CH)
                sum_ps = pmm.tile([P, CH], f32, tag="mm", bufs=2, name="ln_sum")
                sq_ps = pmm.tile([P, CH], f32, tag="mm2", bufs=1, name="ln_sq")
                for i in range(DT):
                    sq = ptr.tile([P, CH], f32r, tag="sq", bufs=3, name="ln_sqt")
                    nc.vector.tensor_tensor(
                        sq[:], src[i][:, cs].bitcast(f32), src[i][:, cs].bitcast(f32),
                        op=Al.mult)
                    nc.tensor.matmul(sum_ps[0:1, :], ones_col[:, :], src[i][:, cs],
                                     start=(i == 0), stop=(i == DT - 1))
                    nc.tensor.matmul(sq_ps[0:1, :], ones_col[:, :], sq[:],
                                     start=(i == 0), stop=(i == DT - 1))
                mu = psm.tile([1, CH], f32, tag="mu", name="ln_mu")
                var = psm.tile([1, CH], f32, tag="var", name="ln_var")
                tmp = psm.tile([1, CH], f32, tag="tmp", name="ln_tmp")
                nc.vector.tensor_scalar_mul(mu[:], sum_ps[0:1, :], invD)
                nc.vector.tensor_scalar_mul(var[:], sq_ps[0:1, :], invD)
                nc.vector.tensor_tensor(tmp[:], mu[:], mu[:], op=Al.mult)
                nc.vector.tensor_tensor(var[:], var[:], tmp[:], op=Al.subtract)
                nc.vector.tensor_scalar_add(var[:], var[:], EPS)
                nc.scalar.sqrt(var[:], var[:])
                srow = psm.tile([1, CH], f32r, tag="srow", name="ln_srow")
                nc.vector.reciprocal(srow[:], var[:])
                so = psm.tile([2, CH], f32r, tag="so", name="ln_so")
                nc.vector.tensor_copy(so[0:2, :], ones_m[0:2, 0:CH])  # row1 stays ones
                nc.vector.tensor_tensor(so[0:1, :], mu[:], srow[:].bitcast(f32),
                                        op=Al.mult)
                nc.vector.tensor_scalar_mul(so[0:1, :], so[0:1, :].bitcast(f32), -1.0)
                sb_ps = pbc.tile([P, CH], f32, tag="bc", bufs=2, name="ln_sb")
                nc.tensor.matmul(sb_ps[:, :], ones_row[:, :], srow[:, :],
                                 start=True, stop=True)
                for i in range(DT):
                    og_ps = pbc.tile([P, CH], f32, tag="bc2", bufs=1, name="ln_og")
                    nc.tensor.matmul(og_ps[:, :], gbx[:, i * P:(i + 1) * P], so[:, :],
                                     start=True, stop=True)
                    t1 = ptr.tile([P, CH], f32, tag="t1", bufs=3, name="ln_t1")
                    nc.vector.tensor_tensor(t1[:], src[i][:, cs].bitcast(f32),
                                            sb_ps[:, :], op=Al.mult)
                    nc.vector.scalar_tensor_tensor(
                        dst[i][:, cs], t1[:], gpx[:, i:i + 1], og_ps[:, :],
                        op0=Al.mult, op1=Al.add)

        # ================= Phase 1: QKV =================
        with tc.tile_pool(name="pxt", bufs=1) as pxt:
            bvb = pxt.tile([P, Dd], f32, name="bvb")
            nc.sync.dma_start(bvb[:], bvb_d[:, :])
            xt_t = []
            for i in range(DT):
                xx = pxt.tile([P, Skv], f32r, name=f"xt{i}")
                nc.sync.dma_start(xx[:], r(xt[i * P:(i + 1) * P, :]))
                xt_t.append(xx)

            with tc.tile_pool(name="pqk", bufs=1) as pqk:
                for j in range(DT):
                    wq = wload(pqk, "wq", 3, f"wq{j}", wqT[:, j * P:(j + 1) * P], DT, P)
                    wk = wload(pqk, "wk", 3, f"wk{j}", wkT[:, j * P:(j + 1) * P], DT, P)
                    for c in range(NCH):
                        ps = pmm.tile([P, CH], f32, tag="mm", bufs=2, name="q_ps")
                        for i in range(DT):
                            nc.tensor.matmul(ps[:, :], wq[:, i * P:(i + 1) * P],
                                             xt_t[i][:, c * CH:(c + 1) * CH],
                                             start=(i == 0), stop=(i == DT - 1))
                        qb = pqk.tile([P, CH], f32r, tag="qb", bufs=2, name="q_bounce")
                        nc.vector.tensor_scalar_add(qb[:], ps[:, :], bq_t[:, j:j + 1])
                        nc.sync.dma_start(
                            qdram[j * P:(j + 1) * P, c * CH:(c + 1) * CH], qb[:])
                    for c in range(SKC):
                        ps = pmm.tile([P, CH], f32, tag="mm", bufs=2, name="k_ps")
                        for i in range(DT):
                            nc.tensor.matmul(ps[:, :], wk[:, i * P:(i + 1) * P],
                                             xt_t[i][:, c * CH:(c + 1) * CH],
                                             start=(i == 0), stop=(i == DT - 1))
                        kb = pqk.tile([P, CH], f32r, tag="kb", bufs=2, name="k_bounce")
                        nc.vector.tensor_scalar_add(kb[:], ps[:, :], bk_t[:, j:j + 1])
                        nc.sync.dma_start(
                            kdram[j * P:(j + 1) * P, c * CH:(c + 1) * CH], kb[:])

            # V (activation-stationary, weight-moving) -> vdram augmented
            HPC = CH // 64  # heads per o-chunk
            with tc.tile_pool(name="pvv", bufs=1) as pvv:
                for oc in range(Dd // CH):
                    wv = wload(pvv, "wv", 2, f"wv{oc}", wvT[:, oc * CH:(oc + 1) * CH],
                               DT, CH)
                    for tt in range(KT):
                        ps = pmm.tile([P, CH], f32, tag="mm", bufs=2, name="v_ps")
                        for i in range(DT):
                            nc.tensor.matmul(ps[:, :], xt_t[i][:, tt * P:(tt + 1) * P],
                                             wv[:, i * CH:(i + 1) * CH],
                                             start=(i == 0), stop=(i == DT - 1))
                        vb = pvv.tile([P, HPC * 65], f32r, tag="vb", bufs=3, name="v_bounce")
                        vb3 = vb[:].rearrange("p (h c) -> p h c", c=65)
                        nc.vector.tensor_copy(
                            vb3[:, :, 64:65],
                            ones_m[:, 0:HPC].unsqueeze(2))
                        nc.vector.tensor_tensor(
                            vb3[:, :, 0:64],
                            ps[:, :].rearrange("p (h c) -> p h c", c=64),
                            bvb[:, oc * CH:(oc + 1) * CH].rearrange("p (h c) -> p h c", c=64),
                            op=Al.add)
                        h0 = oc * HPC
                        nc.sync.dma_start(
                            vdram[h0:h0 + HPC, tt * P:(tt + 1) * P, :].transpose([1, 0, 2]),
                            vb3[:, :, :])

        # ============ Phase 2: attention ============
        es_attn = ExitStack()
        pattn = es_attn.enter_context(
            tc.tile_pool(name="pattn", bufs=1, side="right"))
        if True:
            ctx_t = [pattn.tile([P, T], f32r, tag=f"ctx{j}", name=f"ctx{j}")
                     for j in range(DT)]
            for j in range(Hh // 2):
                qh = pattn.tile([P, T], f32r, tag="qh", bufs=2, name="qh")
                nc.sync.dma_start(qh[:], qdram[j * P:(j + 1) * P, :])
                kh = pattn.tile([P, Skv], f32r, tag="kh", bufs=2, name="kh")
                nc.sync.dma_start(kh[:], kdram[j * P:(j + 1) * P, :])
                vhs = []
                for half in (0, 1):
                    vh = pattn.tile([P, KT * 65], f32r, tag=f"vh{half}",
                                    bufs=2, name=f"vh{half}")
                    vh3 = vh[:].rearrange("p (k c) -> p k c", c=65)
                    nc.sync.dma_start(
                        vh3, vdram[2 * j + half].rearrange("(k p) c -> p k c", p=P))
                    vhs.append(vh3)
                for c in range(NCH):
                    cps = [pmm.tile([P, CH], f32, tag="ctxps", bufs=2,
                                    name=f"ctx_ps{half}") for half in (0, 1)]
                    for kt in range(KT):
                        pts = []
                        for half in (0, 1):
                            hb = half * 64
                            sps = pmm.tile([P, CH], f32, tag="mm", bufs=2, name="s_ps")
                            nc.tensor.matmul(
                                sps[:, :],
                                kh[hb:hb + 64, kt * P:(kt + 1) * P],
                                qh[hb:hb + 64, c * CH:(c + 1) * CH],
                                start=True, stop=True)
                            pt = pattn.tile([P, CH], f32r, tag="pt", bufs=8, name="p_t")
                            nc.scalar.activation(pt[:], sps[:, :], Af.Exp, scale=0.125)
                            pts.append(pt)
                        for half in (0, 1):
                            nc.tensor.matmul(cps[half][0:65, :], vhs[half][:, kt, :],
                                             pts[half][:],
                                             start=(kt == 0), stop=(kt == KT - 1))
                    for half in (0, 1):
                        rec = psm.tile([1, CH], f32r, tag="rec", bufs=2, name="rec")
                        nc.vector.reciprocal(rec[:], cps[half][64:65, :])
                        bc = pbc.tile([P, CH], f32, tag="bc", bufs=2, name="att_bc")
                        nc.tensor.matmul(bc[0:64, :], ones_row[:, 0:64], rec[:, :],
                                         start=True, stop=True)
                        bcs = pattn.tile([P, CH], f32, tag="bcs", bufs=2, name="att_bcs")
                        nc.scalar.copy(bcs[0:64, :], bc[0:64, :])
                        if half == 0:
                            nc.vector.tensor_tensor(
                                ctx_t[j][0:64, c * CH:(c + 1) * CH],
                                cps[half][0:64, :], bcs[0:64, :], op=Al.mult)
                        else:
                            tmp = pattn.tile([P, CH], f32r, tag="ctmp", bufs=2,
                                             name="ctx_tmp")
                            nc.vector.tensor_tensor(
                                tmp[0:64, :], cps[half][0:64, :], bcs[0:64, :],
                                op=Al.mult)
                            nc.sync.dma_start(
                                ctx_t[j][64:128, c * CH:(c + 1) * CH], tmp[0:64, :])

            # ---- out-proj (+ residual & b_out) ----
            es_res = ExitStack()
            pres = es_res.enter_context(tc.tile_pool(name="pres", bufs=1))
            res_t = [pres.tile([P, T], f32r, name=f"res{o}") for o in range(DT)]
            with tc.tile_pool(name="pxr", bufs=1) as pxr:
                xr_t = []
                for o in range(DT):
                    xr = pxr.tile([P, T], f32r, name=f"xr{o}")
                    nc.sync.dma_start(xr[:], r(xt[o * P:(o + 1) * P, 0:T]))
                    xr_t.append(xr)
                for o in range(DT):
                    wo = wload(pattn, "wo", 2, f"wo{o}", woT[:, o * P:(o + 1) * P], DT, P)
                    for c in range(NCH):
                        ps = pmm.tile([P, CH], f32, tag="mm", bufs=2, name="ao_ps")
                        for i in range(DT):
                            nc.tensor.matmul(ps[:, :], wo[:, i * P:(i + 1) * P],
                                             ctx_t[i][:, c * CH:(c + 1) * CH],
                                             start=(i == 0), stop=(i == DT - 1))
                        nc.vector.scalar_tensor_tensor(
                            res_t[o][:, c * CH:(c + 1) * CH], ps[:, :],
                            bo_t[:, o:o + 1],
                            xr_t[o][:, c * CH:(c + 1) * CH].bitcast(f32),
                            op0=Al.add, op1=Al.add)
        es_attn.close()  # ctx/qh/vh/pt freed

        # ============ Phase 3: LN1, gate, MoE, LN2 ============
        es_x1 = ExitStack()
        px1 = es_x1.enter_context(tc.tile_pool(name="px1", bufs=1, side="right"))
        x1_t = [px1.tile([P, T], f32r, name=f"x1_{i}") for i in range(DT)]
        layer_norm(res_t, x1_t, 0, px1)
        es_res.close()

        es_moe = ExitStack()
        pmoe = es_moe.enter_context(tc.tile_pool(name="pmoe", bufs=1))
        pgate = pmoe.tile([P, T], f32r, name="pgate")
        for c in range(NCH):
            gl_ps = pmm.tile([P, CH], f32, tag="mm", bufs=2, name="gl_ps")
            for i in range(DT):
                gw = pmoe.tile([P, Ee], f32r, tag="gw", bufs=3, name=f"gw{c}_{i}")
                nc.sync.dma_start(gw[:], r(gwT[i * P:(i + 1) * P, :]))
                nc.tensor.matmul(gl_ps[0:Ee, :], gw[:],
                                 x1_t[i][:, c * CH:(c + 1) * CH],
                                 start=(i == 0), stop=(i == DT - 1))
            eg = pmoe.tile([P, CH], f32r, tag="eg", bufs=1, name="eg")
            nc.scalar.activation(eg[0:Ee, :], gl_ps[0:Ee, :], Af.Exp,
                                 bias=gb8_t[0:Ee, :])
            gs_ps = pmm.tile([P, CH], f32, tag="mm2", bufs=1, name="gs_ps")
            nc.tensor.matmul(gs_ps[0:1, :], ones_col[0:Ee, :], eg[0:Ee, :],
                             start=True, stop=True)
            grec = psm.tile([1, CH], f32r, tag="rec", bufs=2, name="grec")
            nc.vector.reciprocal(grec[:], gs_ps[0:1, :])
            gb_ps = pbc.tile([P, CH], f32, tag="bc", bufs=2, name="gb_ps")
            nc.tensor.matmul(gb_ps[0:Ee, :], ones_row[:, 0:Ee], grec[:, :],
                             start=True, stop=True)
            nc.vector.tensor_tensor(
                pgate[0:Ee, c * CH:(c + 1) * CH],
                eg[0:Ee, :].bitcast(f32), gb_ps[0:Ee, :], op=Al.mult)

        acc_t = [pmoe.tile([P, T], f32r, tag=f"acc{o}", name=f"acc{o}")
                 for o in range(DT)]
        for e in range(Ee):
            ge_ps = []
            for c in range(NCH):
                grow = pmoe.tile([1, CH], f32r, tag="grow", bufs=2, name=f"grow{e}_{c}")
                nc.sync.dma_start(grow[:], pgate[e:e + 1, c * CH:(c + 1) * CH])
                g = pbc.tile([P, CH], f32, tag="bc", bufs=2, name=f"ge{e}_{c}")
                nc.tensor.matmul(g[:, :], ones_row[:, :], grow[:, :],
                                 start=True, stop=True)
                ge_ps.append(g)
            for o in range(DT):
                we = wload(pmoe, "we", 3, f"we{e}_{o}",
                           ewT[e, :, o * P:(o + 1) * P], DT, P)
                for c in range(NCH):
                    ps = pmm.tile([P, CH], f32, tag="mm", bufs=2, name="moe_ps")
                    for i in range(DT):
                        nc.tensor.matmul(ps[:, :], we[:, i * P:(i + 1) * P],
                                         x1_t[i][:, c * CH:(c + 1) * CH],
                                         start=(i == 0), stop=(i == DT - 1))
                    he = pmoe.tile([P, CH], f32, tag="he", bufs=2, name="he")
                    nc.scalar.activation(he[:], ps[:, :], Af.Relu,
                                         bias=eb_t[:, e * DT + o:e * DT + o + 1])
                    cs = slice(c * CH, (c + 1) * CH)
                    if e == 0:
                        nc.vector.tensor_tensor(
                            acc_t[o][:, cs], he[:], ge_ps[c][:, :], op=Al.mult)
                    else:
                        hg = pmoe.tile([P, CH], f32, tag="hg", bufs=2, name="hg")
                        nc.vector.tensor_tensor(hg[:], he[:], ge_ps[c][:, :],
                                                op=Al.mult)
                        nc.vector.tensor_tensor(
                            acc_t[o][:, cs], acc_t[o][:, cs].bitcast(f32),
                            hg[:], op=Al.add)
        # resid2 = x1 + moe
        for o in range(DT):
            nc.vector.tensor_tensor(acc_t[o][:], acc_t[o][:].bitcast(f32),
                                    x1_t[o][:].bitcast(f32), op=Al.add)
        es_x1.close()

        es_ff = ExitStack()
        pff = es_ff.enter_context(tc.tile_pool(name="pff", bufs=1, side="right"))
        x2_t = [pff.tile([P, T], f32r, tag=f"x2_{i}", name=f"x2_{i}")
                for i in range(DT)]
        layer_norm(acc_t, x2_t, 1, pff)
        es_moe.close()

        # ============ Phase 4: FFN + LN3 ============
        fp_t = [pff.tile([P, T], f32r, tag=f"fp{o}", name=f"fp{o}")
                for o in range(DT)]
        FQ = FT // 4  # f-tiles per FFN quarter
        for fh in range(4):
            h_t = [pff.tile([P, T], f32r, tag=f"h{i2}", name=f"h{fh}_{i2}")
                   for i2 in range(FQ)]
            for o32 in range(FQ):
                o = fh * FQ + o32
                w1 = wload(pff, "w1", 2, f"w1_{o}", w1T[:, o * P:(o + 1) * P], DT, P)
                for c in range(NCH):
                    ps = pmm.tile([P, CH], f32, tag="mm", bufs=2, name="ff1_ps")
                    for i in range(DT):
                        nc.tensor.matmul(ps[:, :], w1[:, i * P:(i + 1) * P],
                                         x2_t[i][:, c * CH:(c + 1) * CH],
                                         start=(i == 0), stop=(i == DT - 1))
                    nc.scalar.activation(h_t[o32][:, c * CH:(c + 1) * CH], ps[:, :],
                                         Af.Relu, bias=b1_t[:, o:o + 1])
            for o in range(DT):
                w2 = wload(pff, "w2", 2, f"w2_{fh}_{o}",
                           w2T[fh * FQ * P:(fh + 1) * FQ * P, o * P:(o + 1) * P],
                           FQ, P)
                for c in range(NCH):
                    ps = pmm.tile([P, CH], f32, tag="mm", bufs=2, name="ff2_ps")
                    for i2 in range(FQ):
                        nc.tensor.matmul(ps[:, :], w2[:, i2 * P:(i2 + 1) * P],
                                         h_t[i2][:, c * CH:(c + 1) * CH],
                                         start=(i2 == 0), stop=(i2 == FQ - 1))
                    cs = slice(c * CH, (c + 1) * CH)
                    if fh == 0:
                        nc.vector.tensor_copy(fp_t[o][:, cs], ps[:, :])
                    elif fh < 3:
                        nc.vector.tensor_tensor(fp_t[o][:, cs],
                                                fp_t[o][:, cs].bitcast(f32),
                                                ps[:, :], op=Al.add)
                    else:
                        nc.vector.scalar_tensor_tensor(
                            fp_t[o][:, cs], ps[:, :], b2_t[:, o:o + 1],
                            fp_t[o][:, cs].bitcast(f32), op0=Al.add, op1=Al.add)
            if fh == 0:
                # fold the residual (x2) into the partial sum
                for o in range(DT):
                    nc.vector.tensor_tensor(fp_t[o][:], fp_t[o][:].bitcast(f32),
                                            x2_t[o][:].bitcast(f32), op=Al.add)
        # LN3 writes into the (now dead) x2 tiles, then out
        layer_norm(fp_t, x2_t, 2, pff)
        for o in range(DT):
            nc.sync.dma_start(out_d[o * P:(o + 1) * P, :], x2_t[o][:].bitcast(f32))
        es_ff.close()

    nc.compile()
    return nc


# ====================== host side ======================

def _pack_col(v, nt):
    # (nt*128,) -> (128, nt) partition-major
    return np.ascontiguousarray(np.asarray(v, np.float32).reshape(nt, P).T)


def make_weight_maps(w_in, b_in, w_out, b_out, gate_w, gate_b, exp_w, exp_b,
                     ffn_w1, ffn_b1, ffn_w2, ffn_b2, g1, be1, g2, be2, g3, be3,
                     cfg):
    Dd, Ee, FT, DT_ = cfg.D, cfg.E, cfg.FT, cfg.DT
    f = np.float32
    ct = np.ascontiguousarray
    m = {
        "wqT": ct(np.asarray(w_in, f)[0:Dd].T),
        "wkT": ct(np.asarray(w_in, f)[Dd:2 * Dd].T),
        "wvT": ct(np.asarray(w_in, f)[2 * Dd:3 * Dd].T),
        "woT": ct(np.asarray(w_out, f).T),
        "gwT": ct(np.asarray(gate_w, f).T),
        "ewT": ct(np.asarray(exp_w, f).transpose(0, 2, 1)),
        "w1T": ct(np.asarray(ffn_w1, f).T),
        "w2T": ct(np.asarray(ffn_w2, f).T),
        "bqp": _pack_col(np.asarray(b_in, f)[0:Dd], DT_),
        "bkp": _pack_col(np.asarray(b_in, f)[Dd:2 * Dd], DT_),
        "bvb": ct(np.broadcast_to(np.asarray(b_in, f)[2 * Dd:3 * Dd], (P, Dd))),
        "bop": _pack_col(b_out, DT_),
        "gb8": np.asarray(gate_b, f).reshape(Ee, 1),
        "ebp": ct(np.asarray(exp_b, f).reshape(Ee * DT_, P).T),
        "b1p": _pack_col(ffn_b1, FT),
        "b2p": _pack_col(ffn_b2, DT_),
        "gba0": ct(np.stack([g1, be1]).astype(f)),
        "gpa0": _pack_col(g1, DT_),
        "gba1": ct(np.stack([g2, be2]).astype(f)),
        "gpa1": _pack_col(g2, DT_),
        "gba2": ct(np.stack([g3, be3]).astype(f)),
        "gpa2": _pack_col(g3, DT_),
    }
    return m


_NC_CACHE = {}


def kernel(x, w_in, b_in, w_out, b_out, gate_w, gate_b, exp_w, exp_b,
           ffn_w1, ffn_b1, ffn_w2, ffn_b2, g1, be1, g2, be2, g3, be3):
    from concourse.bass_utils import run_bass_kernel_spmd

    cfg = FULL_CFG
    x = np.asarray(x, np.float32)
    wm = make_weight_maps(w_in, b_in, w_out, b_out, gate_w, gate_b, exp_w,
                          exp_b, ffn_w1, ffn_b1, ffn_w2, ffn_b2,
                          g1, be1, g2, be2, g3, be3, cfg)
    Th = cfg.T  # tokens per core (one s-half of one batch)
    in_maps = []
    for c in range(NCORES):
        b, half = c // 2, c % 2
        xb = x[:, b, :]                      # (S, D)
        own = xb[half * Th:(half + 1) * Th]  # (T, D)
        other = xb[(1 - half) * Th:(2 - half) * Th]
        xt_c = np.ascontiguousarray(
            np.concatenate([own, other], axis=0).T)  # (D, Skv), own first
        in_maps.append({**wm, "xt": xt_c})

    if "nc" not in _NC_CACHE:
        _NC_CACHE["nc"] = build_program(cfg)
    nc = _NC_CACHE["nc"]

    trace = bool(int(os.environ.get("KERNEL_TRACE", "0")))
    last_exc = None
    for attempt in range(3):
        try:
            res = run_bass_kernel_spmd(nc, in_maps, core_ids=list(range(NCORES)),
                                       trace=trace)
            break
        except Exception as e:  # transient axon/NRT hiccups — retry
            last_exc = e
            if attempt == 2:
                raise
    _NC_CACHE["last_results"] = res

    out = np.empty((S, B, D), np.float32)
    for c in range(NCORES):
        b, half = c // 2, c % 2
        out[half * Th:(half + 1) * Th, b, :] = res.results[c]["out"].T
    return out
